# revision 2
# baseline (speedup 1.0000x reference)
"""TAGConv GNN classifier on 8 Trainium2 NeuronCores.

Sharding: nodes split into 8 contiguous slices (6250/core, padded to 6272);
edges live on the core that owns their dst. The fp16 node table (256B rows)
is allgathered into Shared DRAM each hop; each core gathers its edges' src
rows with super-batched dma_gathers (chunks grouped by dst-group, src-
sorted) and segment-sums them with per-chunk one-hot matmuls on TensorE.
Swapped matmul operands emit the aggregate feature-major directly, so hop
outputs need no PE transpose. Degrees/norms are host-precomputed. Per-graph
readout partials are all-reduced; every core computes identical logits.
"""
import os

os.environ.setdefault("JAX_COMPILATION_CACHE_DIR", "/tmp/jaxcache")
os.environ.setdefault("JAX_PERSISTENT_CACHE_MIN_COMPILE_TIME_SECS", "0")
os.environ.setdefault("JAX_PERSISTENT_CACHE_MIN_ENTRY_SIZE_BYTES", "0")

import numpy as np

import concourse.bass as bass
import concourse.bacc as bacc
import concourse.mybir as mybir
import concourse.tile as tile
from concourse import bass_utils

N, E, G = 50000, 800000, 128
F = 128
CLASSES = 10
HOPS, HLAYERS = 2, 2         # 3 TAGConv layers total
NCORES = 8


def configure(n, e, csup=96):
    """Derived sizes; module-level so debug harnesses can shrink the problem."""
    global N, E, PER, GRP, NPAD, NT, HALF, CSUP
    N, E = n, e
    PER = N // NCORES            # real nodes per core
    GRP = (PER + 127) // 128     # dst groups of 128 per core
    NPAD = GRP * 128             # padded nodes per core
    NT = NCORES * NPAD           # padded total
    HALF = NT // 2               # int16-safe split of the node table
    CSUP = csup                  # max chunks per super-gather


configure(N, E)

F16 = mybir.dt.float16
FP = mybir.dt.float32
I16 = mybir.dt.int16

NW = (HLAYERS + 1) * F + CLASSES   # packed weight columns


def _prep_edges(src, dst, norm):
    """Chunk edges by (dst-group, table-half), src-sorted, batched into
    supers of whole groups with A-chunks then B-chunks contiguous.
    nslot carries norm[dst] per edge (0 on pads) so the one-hot can be
    pre-scaled by the dst-side norm."""
    src = np.asarray(src).astype(np.int64)
    dst = np.asarray(dst).astype(np.int64)
    core = dst // PER
    local = dst - core * PER
    grp = local // 128
    slot = local % 128
    ps = (src // PER) * NPAD + (src % PER)      # padded global src id
    hi = (ps >= HALF).astype(np.int64)

    key = (core * GRP + grp) * 2 + hi
    order = np.lexsort((ps, key))
    cnt = np.bincount(key, minlength=NCORES * GRP * 2).reshape(NCORES, GRP, 2)
    CA = (-(-cnt[:, :, 0].max(axis=0) // 128)).astype(int)   # per-group chunks
    CB = (-(-cnt[:, :, 1].max(axis=0) // 128)).astype(int)

    # supers: consecutive whole groups, <= CSUP chunks each
    supers = []   # (c0, nchA, nchB, [(g, a0, na, b0, nb)]) ; a0/b0 local
    i = 0
    c0 = 0
    while i < GRP:
        j, tot = i, 0
        while j < GRP and tot + CA[j] + CB[j] <= CSUP:
            tot += CA[j] + CB[j]
            j += 1
        assert j > i, f"group {i} exceeds CSUP={CSUP}"
        gl, a, nchA = [], 0, int(sum(CA[i:j]))
        b = nchA
        for g in range(i, j):
            gl.append((g, a, int(CA[g]), b, int(CB[g])))
            a += int(CA[g])
            b += int(CB[g])
        supers.append((c0, nchA, int(sum(CB[i:j])), gl))
        c0 += tot
        i = j
    NCH = c0
    TOT = NCH * 128

    # global chunk offset per (group, half) following the super layout
    choff = np.zeros((GRP, 2), int)
    for (c0s, nchA, nchB, gl) in supers:
        for (g, a0, na, b0, nb) in gl:
            choff[g, 0] = c0s + a0
            choff[g, 1] = c0s + b0

    gidx = np.zeros((NCORES, TOT), np.int16)
    slotv = np.full((NCORES, TOT), -1.0, np.float16)
    nslotv = np.zeros((NCORES, TOT), np.float16)
    sp = ps[order]
    ss = slot[order]
    sn = norm[dst[order]]
    starts = np.concatenate([[0], np.cumsum(cnt.reshape(-1))]).astype(int)
    for c in range(NCORES):
        for g in range(GRP):
            for h in range(2):
                k = (c * GRP + g) * 2 + h
                n = int(cnt[c, g, h])
                s0 = starts[k]
                off = choff[g, h] * 128
                gidx[c, off : off + n] = (sp[s0 : s0 + n] - h * HALF).astype(np.int16)
                slotv[c, off : off + n] = ss[s0 : s0 + n]
                nslotv[c, off : off + n] = sn[s0 : s0 + n]

    gs = np.ascontiguousarray(gidx.reshape(NCORES, -1, 16).transpose(0, 2, 1))
    slot_cols = np.ascontiguousarray(
        slotv.reshape(NCORES, NCH, 128).transpose(0, 2, 1))
    nslot_cols = np.ascontiguousarray(
        nslotv.reshape(NCORES, NCH, 128).transpose(0, 2, 1))
    return gs, slot_cols, nslot_cols, supers, NCH, TOT


def _build_program(supers, NCH, TOT, shared_tables=True):
    W16 = TOT // 16
    nc = bacc.Bacc("TRN2", target_bir_lowering=False, debug=False,
                   num_devices=NCORES)
    RG = [list(range(NCORES))]
    aspace = "Shared" if shared_tables else "Local"

    x_d = nc.dram_tensor("x_h", [NPAD, F], F16, kind="ExternalInput")
    gs_d = nc.dram_tensor("gs_idx", [16, W16], I16, kind="ExternalInput")
    slot_d = nc.dram_tensor("slot_h", [128, NCH], F16, kind="ExternalInput")
    nslot_d = nc.dram_tensor("nslot_h", [128, NCH], F16, kind="ExternalInput")
    colf_d = nc.dram_tensor("colf", [128, 2 * GRP + 3 + CLASSES], FP,
                            kind="ExternalInput")
    wpk_d = nc.dram_tensor("wpk", [(HOPS + 1) * F, NW], F16, kind="ExternalInput")
    out_d = nc.dram_tensor("out", [G, CLASSES], FP, kind="ExternalOutput")

    mul = mybir.AluOpType.mult

    with tile.TileContext(nc) as tc:
        with (
            tc.tile_pool(name="const", bufs=1) as cp,
            tc.tile_pool(name="work", bufs=2) as wp,
            tc.tile_pool(name="psmm", bufs=3, space="PSUM") as pmm,
            tc.tile_pool(name="pstr", bufs=2, space="PSUM") as ptr,
            tc.tile_pool(name="psro", bufs=2, space="PSUM") as pro,
            tc.tile_pool(name="dram", bufs=1, space="DRAM") as dp,
        ):
            gidx_t = cp.tile([128, W16], I16)
            slot_t = cp.tile([128, NCH], F16)
            nslot_t = cp.tile([128, NCH], F16)
            colf_t = cp.tile([128, 2 * GRP + 3 + CLASSES], FP)
            wall_t = cp.tile([128, HOPS + 1, NW], F16)
            normh_t = cp.tile([128, GRP], F16)
            gsloth_t = cp.tile([128, GRP], F16)
            validh_t = cp.tile([128, GRP], F16)
            iotah_t = cp.tile([128, 128], F16)
            identh_t = cp.tile([128, 128], F16)
            og_t = cp.tile([128, GRP, 128], F16)
            xt = cp.tile([128, GRP, F], F16)
            tn_t = cp.tile([128, GRP, F], F16)
            f0T = cp.tile([128, NPAD], F16)
            f1T = cp.tile([128, NPAD], F16)
            f2T = cp.tile([128, NPAD], F16)
            ro2_t = cp.tile([128, F + 1], FP)
            cnt_t = cp.tile([128, 1], FP)
            rcp_t = cp.tile([128, 1], FP)
            hgh_t = cp.tile([128, F], F16)
            hgT_t = cp.tile([128, 128], F16)
            logit_t = cp.tile([128, CLASSES], FP)

            # Shared DRAM tensors may be written by only one instruction:
            # one table per AllGather (t0 + per-layer hop1 + 2 layer outs).
            Ts = [dp.tile([NT, F], F16, addr_space=aspace, name=f"T{i}")
                  for i in range(2 * HLAYERS + HOPS)]
            ag_in = dp.tile([NPAD, F], F16)
            ar_in = dp.tile([128, F + 1], FP)
            ar_out = dp.tile([128, F + 1], FP)

            rearr = lambda ap: ap.rearrange("(g p) f -> p g f", p=128)

            # ---- constants ----
            for k in range(8):
                nc.sync.dma_start(gidx_t[16 * k : 16 * (k + 1), :], gs_d[:, :])
            nc.sync.dma_start(slot_t[:], slot_d[:, :])
            nc.sync.dma_start(nslot_t[:], nslot_d[:, :])
            nc.sync.dma_start(colf_t[:], colf_d[:, :])
            for k in range(HOPS + 1):
                nc.sync.dma_start(wall_t[:, k, :], wpk_d[k * 128 : (k + 1) * 128, :])
            nc.sync.dma_start(xt[:], rearr(x_d[:, :]))

            normc = colf_t[:, 0:GRP]
            gslot = colf_t[:, GRP : 2 * GRP]
            b_t = colf_t[:, 2 * GRP : 2 * GRP + 3]
            bcr = colf_t[:, 2 * GRP + 3 : 2 * GRP + 3 + CLASSES]

            nc.gpsimd.iota(iotah_t[:], pattern=[[1, 128]], base=0,
                           channel_multiplier=0,
                           allow_small_or_imprecise_dtypes=True)
            icolh_t = cp.tile([128, 1], F16)
            nc.gpsimd.iota(icolh_t[:], pattern=[[0, 1]], base=0,
                           channel_multiplier=1,
                           allow_small_or_imprecise_dtypes=True)
            nc.vector.tensor_tensor(identh_t[:],
                                    icolh_t[:].broadcast_to([128, 128]),
                                    iotah_t[:], mybir.AluOpType.is_equal)
            nc.vector.tensor_copy(normh_t[:], normc)
            nc.vector.tensor_copy(gsloth_t[:], gslot)
            nc.vector.tensor_scalar_add(validh_t[:], gsloth_t[:], 1.0)
            nc.vector.tensor_scalar_min(validh_t[:], validh_t[:], 1.0)
            nc.vector.tensor_tensor(
                og_t[:],
                gsloth_t[:].unsqueeze(2).broadcast_to([128, GRP, 128]),
                iotah_t[:].unsqueeze(1).broadcast_to([128, GRP, 128]),
                mybir.AluOpType.is_equal)

            def hop(Tsrc, fT, make_table):
                """gather supers -> one-hot segment-sum (feature-major) -> fT;
                optionally also stage the next node table into tn_t."""
                for (c0, nchA, nchB, gl) in supers:
                    nch = nchA + nchB
                    vb = wp.tile([128, CSUP, F], F16, name="vb", tag="vb")
                    if nchA:
                        nc.gpsimd.dma_gather(
                            vb[:, 0:nchA, :], Tsrc[:, :],
                            gidx_t[:, c0 * 8 : (c0 + nchA) * 8],
                            nchA * 128, nchA * 128, F, single_packet=False)
                    if nchB:
                        nc.gpsimd.dma_gather(
                            vb[:, nchA:nch, :], Tsrc[HALF:, :],
                            gidx_t[:, (c0 + nchA) * 8 : (c0 + nch) * 8],
                            nchB * 128, nchB * 128, F, single_packet=False)
                    oh = wp.tile([128, CSUP, 128], F16, name="oh", tag="oh")
                    nc.vector.tensor_tensor(
                        oh[:, 0:nch, :],
                        slot_t[:, c0 : c0 + nch].unsqueeze(2)
                              .broadcast_to([128, nch, 128]),
                        iotah_t[:].unsqueeze(1).broadcast_to([128, nch, 128]),
                        mybir.AluOpType.is_equal)
                    nc.vector.tensor_tensor(
                        oh[:, 0:nch, :], oh[:, 0:nch, :],
                        nslot_t[:, c0 : c0 + nch].unsqueeze(2)
                               .broadcast_to([128, nch, 128]), mul)
                    for (g, a0, na, b0, nb) in gl:
                        gsl = slice(g * 128, (g + 1) * 128)
                        chunks = list(range(a0, a0 + na)) + \
                                 list(range(b0, b0 + nb))
                        if not chunks:
                            nc.vector.memset(fT[:, gsl], 0.0)
                            continue
                        psT = pmm.tile([128, 128], FP, name="psT", tag="mm")
                        for ci, c in enumerate(chunks):
                            nc.tensor.matmul(psT[:], vb[:, c, :], oh[:, c, :],
                                             start=(ci == 0),
                                             stop=(ci == len(chunks) - 1))
                        nc.vector.tensor_copy(fT[:, gsl], psT[:])
                if make_table:
                    for g in range(GRP):
                        gsl = slice(g * 128, (g + 1) * 128)
                        pt = ptr.tile([128, 128], F16, name="pt", tag="tr")
                        nc.tensor.transpose(pt[:], fT[:, gsl], identh_t[:])
                        nc.vector.tensor_tensor(
                            tn_t[:, g, :], pt[:],
                            normh_t[:, g : g + 1].broadcast_to([128, F]), mul)

            def emit_table(Tdst):
                nc.sync.dma_start(rearr(ag_in[:, :]), tn_t[:])
                nc.gpsimd.collective_compute(
                    "AllGather", mybir.AluOpType.bypass, replica_groups=RG,
                    ins=[ag_in[:, :].opt()], outs=[Tdst[:, :].opt()])

            # ---- t0: table = x*norm, f0T = x^T ----
            nc.vector.tensor_tensor(
                tn_t[:], xt[:],
                normh_t[:].unsqueeze(2).broadcast_to([128, GRP, F]), mul)
            emit_table(Ts[0])
            for g in range(GRP):
                gsl = slice(g * 128, (g + 1) * 128)
                pt = ptr.tile([128, 128], F16, name="pt0", tag="tr")
                nc.tensor.transpose(pt[:], xt[:, g, :], identh_t[:])
                nc.vector.tensor_copy(f0T[:, gsl], pt[:])

            pr = None
            for l in range(HLAYERS + 1):
                hop(Ts[2 * l], f1T, make_table=True)
                emit_table(Ts[2 * l + 1])
                hop(Ts[2 * l + 1], f2T, make_table=False)
                for g in range(GRP):
                    gsl = slice(g * 128, (g + 1) * 128)
                    ph = pmm.tile([128, 128], FP, name="ph", tag="mm")
                    for k, fk in enumerate((f0T, f1T, f2T)):
                        nc.tensor.matmul(ph[:], wall_t[:, k, l * F : (l + 1) * F],
                                         fk[:, gsl], start=(k == 0), stop=(k == 2))
                    nc.scalar.activation(f0T[:, gsl], ph[:],
                                         mybir.ActivationFunctionType.Relu,
                                         bias=b_t[:, l : l + 1])
                if l < HLAYERS:
                    for g in range(GRP):
                        gsl = slice(g * 128, (g + 1) * 128)
                        pt2 = ptr.tile([128, 128], F16, name="pt2", tag="tr")
                        nc.tensor.transpose(pt2[:], f0T[:, gsl], identh_t[:])
                        nc.vector.tensor_tensor(
                            tn_t[:, g, :], pt2[:],
                            normh_t[:, g : g + 1].broadcast_to([128, F]), mul)
                    emit_table(Ts[2 * l + 2])
                else:
                    pr = pro.tile([128, F + 1], FP, name="pr", tag="ro")
                    for g in range(GRP):
                        gsl = slice(g * 128, (g + 1) * 128)
                        pt3 = ptr.tile([128, 128], F16, name="pt3", tag="tr")
                        nc.tensor.transpose(pt3[:], f0T[:, gsl], identh_t[:])
                        rr = wp.tile([128, F + 1], F16, name="rr", tag="rr")
                        nc.vector.tensor_copy(rr[:, 0:F], pt3[:])
                        nc.vector.tensor_copy(rr[:, F : F + 1],
                                              validh_t[:, g : g + 1])
                        nc.tensor.matmul(pr[:], og_t[:, g, :], rr[:],
                                         start=(g == 0), stop=(g == GRP - 1))

            # ---- readout: all-reduce partial sums, mean, classify ----
            ro_t = cp.tile([128, F + 1], FP)
            nc.vector.tensor_copy(ro_t[:], pr[:])
            nc.sync.dma_start(ar_in[:, :], ro_t[:])
            nc.gpsimd.collective_compute(
                "AllReduce", mybir.AluOpType.add, replica_groups=RG,
                ins=[ar_in[:, :].opt()], outs=[ar_out[:, :].opt()])
            nc.sync.dma_start(ro2_t[:], ar_out[:, :])
            nc.vector.tensor_scalar_max(cnt_t[:], ro2_t[:, F : F + 1], 1.0)
            nc.vector.reciprocal(rcp_t[:], cnt_t[:])
            nc.vector.tensor_tensor(hgh_t[:], ro2_t[:, 0:F],
                                    rcp_t[:].broadcast_to([128, F]), mul)
            ptf = ptr.tile([128, 128], F16, name="ptf", tag="tr")
            nc.tensor.transpose(ptf[:], hgh_t[:], identh_t[:])
            nc.vector.tensor_copy(hgT_t[:], ptf[:])
            plog = pro.tile([128, CLASSES], FP, name="plog", tag="ro")
            nc.tensor.matmul(plog[:], hgT_t[:],
                             wall_t[:, 0, (HLAYERS + 1) * F :],
                             start=True, stop=True)
            nc.vector.tensor_tensor(logit_t[:], plog[:], bcr,
                                    mybir.AluOpType.add)
            nc.sync.dma_start(out_d[:, :], logit_t[:])

    nc.finalize()
    return nc


def _make_in_maps(x, src, dst, graph_ids, Ws, bs, Wc, bc):
    deg = np.bincount(np.asarray(dst).astype(np.int64), minlength=N)
    norm = np.where(deg < 1, 1.0, deg).astype(np.float32) ** -0.5
    gs, slot_cols, nslot_cols, supers, NCH, TOT = _prep_edges(src, dst, norm)

    x = np.asarray(x, np.float32)
    graph_ids = np.asarray(graph_ids, np.int64)
    wpk = np.zeros(((HOPS + 1) * F, NW), np.float16)
    for l in range(HLAYERS + 1):
        wpk[:, l * F : (l + 1) * F] = np.asarray(Ws[l], np.float32)
    wpk[0:F, (HLAYERS + 1) * F :] = np.asarray(Wc, np.float32)
    bcol = np.stack([np.asarray(b, np.float32) for b in bs], 1)
    bcr = np.tile(np.asarray(bc, np.float32)[None, :], (128, 1))

    in_maps = []
    for c in range(NCORES):
        xl = np.zeros((NPAD, F), np.float16)
        xl[:PER] = x[c * PER : (c + 1) * PER]
        nrm = np.ones(NPAD, np.float32)
        nrm[:PER] = norm[c * PER : (c + 1) * PER]
        gsl = np.full(NPAD, -1.0, np.float32)
        gsl[:PER] = graph_ids[c * PER : (c + 1) * PER]
        colf = np.zeros((128, 2 * GRP + 3 + CLASSES), np.float32)
        colf[:, 0:GRP] = nrm.reshape(GRP, 128).T
        colf[:, GRP : 2 * GRP] = gsl.reshape(GRP, 128).T
        colf[:, 2 * GRP : 2 * GRP + 3] = bcol
        colf[:, 2 * GRP + 3 :] = bcr
        in_maps.append(dict(x_h=xl, gs_idx=gs[c], slot_h=slot_cols[c],
                            nslot_h=nslot_cols[c], colf=colf, wpk=wpk))
    return in_maps, supers, NCH, TOT


def kernel(x, src, dst, graph_ids, W0, b0, W1, b1, W2, b2, Wc, bc, **_):
    in_maps, supers, NCH, TOT = _make_in_maps(
        x, src, dst, graph_ids, [W0, W1, W2], [b0, b1, b2], Wc, bc)
    nc = _build_program(supers, NCH, TOT)
    res = bass_utils.run_bass_kernel_spmd(nc, in_maps, core_ids=list(range(NCORES)))
    return np.asarray(res.results[0]["out"], np.float32)


# revision 3
# speedup vs baseline: 2.4124x; 2.4124x over previous
"""TAGConv GNN classifier on 8 Trainium2 NeuronCores.

Sharding: nodes split into 8 contiguous slices (6250/core, padded to 6272);
edges live on the core that owns their dst. The fp16 node table (256B rows)
is allgathered into Shared DRAM each hop; each core gathers its edges' src
rows with super-batched dma_gathers (chunks grouped by dst-group, src-
sorted) and segment-sums them with per-chunk one-hot matmuls on TensorE.
Swapped matmul operands emit the aggregate feature-major directly, so hop
outputs need no PE transpose. Degrees/norms are host-precomputed. Per-graph
readout partials are all-reduced; every core computes identical logits.
"""
import os

os.environ.setdefault("JAX_COMPILATION_CACHE_DIR", "/tmp/jaxcache")
os.environ.setdefault("JAX_PERSISTENT_CACHE_MIN_COMPILE_TIME_SECS", "0")
os.environ.setdefault("JAX_PERSISTENT_CACHE_MIN_ENTRY_SIZE_BYTES", "0")

import numpy as np

import concourse.bass as bass
import concourse.bacc as bacc
import concourse.mybir as mybir
import concourse.tile as tile
from concourse import bass_utils

# The env defaults above only help if jax wasn't imported yet; config.update
# works either way, making warm invocations hit the persistent compile cache.
try:
    import jax

    jax.config.update("jax_compilation_cache_dir",
                      os.environ["JAX_COMPILATION_CACHE_DIR"])
    jax.config.update("jax_persistent_cache_min_compile_time_secs", 0.0)
    jax.config.update("jax_persistent_cache_min_entry_size_bytes", 0)
except Exception:
    pass

N, E, G = 50000, 800000, 128
F = 128
CLASSES = 10
HOPS, HLAYERS = 2, 2         # 3 TAGConv layers total
NCORES = 8


def configure(n, e, csup=96):
    """Derived sizes; module-level so debug harnesses can shrink the problem."""
    global N, E, PER, GRP, NPAD, NT, HALF, CSUP
    N, E = n, e
    PER = N // NCORES            # real nodes per core
    GRP = (PER + 127) // 128     # dst groups of 128 per core
    NPAD = GRP * 128             # padded nodes per core
    NT = NCORES * NPAD           # padded total
    HALF = NT // 2               # int16-safe split of the node table
    CSUP = csup                  # max chunks per super-gather


configure(N, E)

F16 = mybir.dt.float16
FP = mybir.dt.float32
I16 = mybir.dt.int16

NW = (HLAYERS + 1) * F + CLASSES   # packed weight columns


def _prep_edges(src, dst, norm):
    """Chunk edges by (dst-group, table-half), src-sorted, batched into
    supers of whole groups with A-chunks then B-chunks contiguous.
    nslot carries norm[dst] per edge (0 on pads) so the one-hot can be
    pre-scaled by the dst-side norm."""
    src = np.asarray(src).astype(np.int64)
    dst = np.asarray(dst).astype(np.int64)
    core = dst // PER
    local = dst - core * PER
    grp = local // 128
    slot = local % 128
    ps = (src // PER) * NPAD + (src % PER)      # padded global src id
    hi = (ps >= HALF).astype(np.int64)

    key = (core * GRP + grp) * 2 + hi
    order = np.lexsort((ps, key))
    cnt = np.bincount(key, minlength=NCORES * GRP * 2).reshape(NCORES, GRP, 2)
    CA = (-(-cnt[:, :, 0].max(axis=0) // 128)).astype(int)   # per-group chunks
    CB = (-(-cnt[:, :, 1].max(axis=0) // 128)).astype(int)

    # supers: consecutive whole groups, <= CSUP chunks each
    supers = []   # (c0, nchA, nchB, [(g, a0, na, b0, nb)]) ; a0/b0 local
    i = 0
    c0 = 0
    while i < GRP:
        j, tot = i, 0
        while j < GRP and tot + CA[j] + CB[j] <= CSUP:
            tot += CA[j] + CB[j]
            j += 1
        assert j > i, f"group {i} exceeds CSUP={CSUP}"
        gl, a, nchA = [], 0, int(sum(CA[i:j]))
        b = nchA
        for g in range(i, j):
            gl.append((g, a, int(CA[g]), b, int(CB[g])))
            a += int(CA[g])
            b += int(CB[g])
        supers.append((c0, nchA, int(sum(CB[i:j])), gl))
        c0 += tot
        i = j
    NCH = c0
    TOT = NCH * 128

    # global chunk offset per (group, half) following the super layout
    choff = np.zeros((GRP, 2), int)
    for (c0s, nchA, nchB, gl) in supers:
        for (g, a0, na, b0, nb) in gl:
            choff[g, 0] = c0s + a0
            choff[g, 1] = c0s + b0

    gidx = np.zeros((NCORES, TOT), np.int16)
    slotv = np.full((NCORES, TOT), -1.0, np.float16)
    nslotv = np.zeros((NCORES, TOT), np.float16)
    sp = ps[order]
    ss = slot[order]
    sn = norm[dst[order]]
    starts = np.concatenate([[0], np.cumsum(cnt.reshape(-1))]).astype(int)
    for c in range(NCORES):
        for g in range(GRP):
            for h in range(2):
                k = (c * GRP + g) * 2 + h
                n = int(cnt[c, g, h])
                s0 = starts[k]
                off = choff[g, h] * 128
                gidx[c, off : off + n] = (sp[s0 : s0 + n] - h * HALF).astype(np.int16)
                slotv[c, off : off + n] = ss[s0 : s0 + n]
                nslotv[c, off : off + n] = sn[s0 : s0 + n]

    gs = np.ascontiguousarray(gidx.reshape(NCORES, -1, 16).transpose(0, 2, 1))
    slot_cols = np.ascontiguousarray(
        slotv.reshape(NCORES, NCH, 128).transpose(0, 2, 1))
    nslot_cols = np.ascontiguousarray(
        nslotv.reshape(NCORES, NCH, 128).transpose(0, 2, 1))
    return gs, slot_cols, nslot_cols, supers, NCH, TOT


def _build_program(supers, NCH, TOT, shared_tables=True):
    W16 = TOT // 16
    nc = bacc.Bacc("TRN2", target_bir_lowering=False, debug=False,
                   num_devices=NCORES)
    RG = [list(range(NCORES))]
    aspace = "Shared" if shared_tables else "Local"

    x_d = nc.dram_tensor("x_h", [NPAD, F], F16, kind="ExternalInput")
    gs_d = nc.dram_tensor("gs_idx", [16, W16], I16, kind="ExternalInput")
    slot_d = nc.dram_tensor("slot_h", [128, NCH], F16, kind="ExternalInput")
    nslot_d = nc.dram_tensor("nslot_h", [128, NCH], F16, kind="ExternalInput")
    colf_d = nc.dram_tensor("colf", [128, 2 * GRP + 3 + CLASSES], FP,
                            kind="ExternalInput")
    wpk_d = nc.dram_tensor("wpk", [(HOPS + 1) * F, NW], F16, kind="ExternalInput")
    out_d = nc.dram_tensor("out", [G, CLASSES], FP, kind="ExternalOutput")

    mul = mybir.AluOpType.mult

    with tile.TileContext(nc) as tc:
        with (
            tc.tile_pool(name="const", bufs=1) as cp,
            tc.tile_pool(name="work", bufs=2) as wp,
            tc.tile_pool(name="psmm", bufs=3, space="PSUM") as pmm,
            tc.tile_pool(name="pstr", bufs=2, space="PSUM") as ptr,
            tc.tile_pool(name="psro", bufs=2, space="PSUM") as pro,
            tc.tile_pool(name="dram", bufs=1, space="DRAM") as dp,
        ):
            gidx_t = cp.tile([128, W16], I16)
            slot_t = cp.tile([128, NCH], F16)
            nslot_t = cp.tile([128, NCH], F16)
            colf_t = cp.tile([128, 2 * GRP + 3 + CLASSES], FP)
            wall_t = cp.tile([128, HOPS + 1, NW], F16)
            normh_t = cp.tile([128, GRP], F16)
            gsloth_t = cp.tile([128, GRP], F16)
            validh_t = cp.tile([128, GRP], F16)
            iotah_t = cp.tile([128, 128], F16)
            identh_t = cp.tile([128, 128], F16)
            og_t = cp.tile([128, GRP, 128], F16)
            xt = cp.tile([128, GRP, F], F16)
            tn_t = cp.tile([128, GRP, F], F16)
            f0T = cp.tile([128, NPAD], F16)
            f1T = cp.tile([128, NPAD], F16)
            f2T = cp.tile([128, NPAD], F16)
            ro2_t = cp.tile([128, F + 1], FP)
            cnt_t = cp.tile([128, 1], FP)
            rcp_t = cp.tile([128, 1], FP)
            hgh_t = cp.tile([128, F], F16)
            hgT_t = cp.tile([128, 128], F16)
            logit_t = cp.tile([128, CLASSES], FP)

            # Shared DRAM tensors may be written by only one instruction:
            # one table per AllGather (t0 + per-layer hop1 + 2 layer outs).
            Ts = [dp.tile([NT, F], F16, addr_space=aspace, name=f"T{i}")
                  for i in range(2 * HLAYERS + HOPS)]
            ag_in = dp.tile([NPAD, F], F16)
            ar_in = dp.tile([128, F + 1], FP)
            ar_out = dp.tile([128, F + 1], FP)

            rearr = lambda ap: ap.rearrange("(g p) f -> p g f", p=128)

            # ---- constants ----
            for k in range(8):
                nc.sync.dma_start(gidx_t[16 * k : 16 * (k + 1), :], gs_d[:, :])
            nc.sync.dma_start(slot_t[:], slot_d[:, :])
            nc.sync.dma_start(nslot_t[:], nslot_d[:, :])
            nc.sync.dma_start(colf_t[:], colf_d[:, :])
            for k in range(HOPS + 1):
                nc.sync.dma_start(wall_t[:, k, :], wpk_d[k * 128 : (k + 1) * 128, :])
            nc.sync.dma_start(xt[:], rearr(x_d[:, :]))

            normc = colf_t[:, 0:GRP]
            gslot = colf_t[:, GRP : 2 * GRP]
            b_t = colf_t[:, 2 * GRP : 2 * GRP + 3]
            bcr = colf_t[:, 2 * GRP + 3 : 2 * GRP + 3 + CLASSES]

            nc.gpsimd.iota(iotah_t[:], pattern=[[1, 128]], base=0,
                           channel_multiplier=0,
                           allow_small_or_imprecise_dtypes=True)
            icolh_t = cp.tile([128, 1], F16)
            nc.gpsimd.iota(icolh_t[:], pattern=[[0, 1]], base=0,
                           channel_multiplier=1,
                           allow_small_or_imprecise_dtypes=True)
            nc.vector.tensor_tensor(identh_t[:],
                                    icolh_t[:].broadcast_to([128, 128]),
                                    iotah_t[:], mybir.AluOpType.is_equal)
            nc.vector.tensor_copy(normh_t[:], normc)
            nc.vector.tensor_copy(gsloth_t[:], gslot)
            nc.vector.tensor_scalar_add(validh_t[:], gsloth_t[:], 1.0)
            nc.vector.tensor_scalar_min(validh_t[:], validh_t[:], 1.0)
            nc.vector.tensor_tensor(
                og_t[:],
                gsloth_t[:].unsqueeze(2).broadcast_to([128, GRP, 128]),
                iotah_t[:].unsqueeze(1).broadcast_to([128, GRP, 128]),
                mybir.AluOpType.is_equal)

            def hop(Tsrc, fT, make_table):
                """gather supers -> one-hot segment-sum (feature-major) -> fT;
                optionally also stage the next node table into tn_t."""
                for (c0, nchA, nchB, gl) in supers:
                    nch = nchA + nchB
                    vb = wp.tile([128, CSUP, F], F16, name="vb", tag="vb")
                    if nchA:
                        nc.gpsimd.dma_gather(
                            vb[:, 0:nchA, :], Tsrc[:, :],
                            gidx_t[:, c0 * 8 : (c0 + nchA) * 8],
                            nchA * 128, nchA * 128, F, single_packet=False)
                    if nchB:
                        nc.gpsimd.dma_gather(
                            vb[:, nchA:nch, :], Tsrc[HALF:, :],
                            gidx_t[:, (c0 + nchA) * 8 : (c0 + nch) * 8],
                            nchB * 128, nchB * 128, F, single_packet=False)
                    oh = wp.tile([128, CSUP, 128], F16, name="oh", tag="oh")
                    nc.vector.tensor_tensor(
                        oh[:, 0:nch, :],
                        slot_t[:, c0 : c0 + nch].unsqueeze(2)
                              .broadcast_to([128, nch, 128]),
                        iotah_t[:].unsqueeze(1).broadcast_to([128, nch, 128]),
                        mybir.AluOpType.is_equal)
                    nc.vector.tensor_tensor(
                        oh[:, 0:nch, :], oh[:, 0:nch, :],
                        nslot_t[:, c0 : c0 + nch].unsqueeze(2)
                               .broadcast_to([128, nch, 128]), mul)
                    for (g, a0, na, b0, nb) in gl:
                        gsl = slice(g * 128, (g + 1) * 128)
                        chunks = list(range(a0, a0 + na)) + \
                                 list(range(b0, b0 + nb))
                        if not chunks:
                            nc.vector.memset(fT[:, gsl], 0.0)
                            continue
                        psT = pmm.tile([128, 128], FP, name="psT", tag="mm")
                        for ci, c in enumerate(chunks):
                            nc.tensor.matmul(psT[:], vb[:, c, :], oh[:, c, :],
                                             start=(ci == 0),
                                             stop=(ci == len(chunks) - 1))
                        nc.vector.tensor_copy(fT[:, gsl], psT[:])
                if make_table:
                    for g in range(GRP):
                        gsl = slice(g * 128, (g + 1) * 128)
                        pt = ptr.tile([128, 128], F16, name="pt", tag="tr")
                        nc.tensor.transpose(pt[:], fT[:, gsl], identh_t[:])
                        nc.vector.tensor_tensor(
                            tn_t[:, g, :], pt[:],
                            normh_t[:, g : g + 1].broadcast_to([128, F]), mul)

            def emit_table(Tdst):
                nc.sync.dma_start(rearr(ag_in[:, :]), tn_t[:])
                nc.gpsimd.collective_compute(
                    "AllGather", mybir.AluOpType.bypass, replica_groups=RG,
                    ins=[ag_in[:, :].opt()], outs=[Tdst[:, :].opt()])

            # ---- t0: table = x*norm, f0T = x^T ----
            nc.vector.tensor_tensor(
                tn_t[:], xt[:],
                normh_t[:].unsqueeze(2).broadcast_to([128, GRP, F]), mul)
            emit_table(Ts[0])
            for g in range(GRP):
                gsl = slice(g * 128, (g + 1) * 128)
                pt = ptr.tile([128, 128], F16, name="pt0", tag="tr")
                nc.tensor.transpose(pt[:], xt[:, g, :], identh_t[:])
                nc.vector.tensor_copy(f0T[:, gsl], pt[:])

            pr = None
            for l in range(HLAYERS + 1):
                hop(Ts[2 * l], f1T, make_table=True)
                emit_table(Ts[2 * l + 1])
                hop(Ts[2 * l + 1], f2T, make_table=False)
                for g in range(GRP):
                    gsl = slice(g * 128, (g + 1) * 128)
                    ph = pmm.tile([128, 128], FP, name="ph", tag="mm")
                    for k, fk in enumerate((f0T, f1T, f2T)):
                        nc.tensor.matmul(ph[:], wall_t[:, k, l * F : (l + 1) * F],
                                         fk[:, gsl], start=(k == 0), stop=(k == 2))
                    nc.scalar.activation(f0T[:, gsl], ph[:],
                                         mybir.ActivationFunctionType.Relu,
                                         bias=b_t[:, l : l + 1])
                if l < HLAYERS:
                    for g in range(GRP):
                        gsl = slice(g * 128, (g + 1) * 128)
                        pt2 = ptr.tile([128, 128], F16, name="pt2", tag="tr")
                        nc.tensor.transpose(pt2[:], f0T[:, gsl], identh_t[:])
                        nc.vector.tensor_tensor(
                            tn_t[:, g, :], pt2[:],
                            normh_t[:, g : g + 1].broadcast_to([128, F]), mul)
                    emit_table(Ts[2 * l + 2])
                else:
                    pr = pro.tile([128, F + 1], FP, name="pr", tag="ro")
                    for g in range(GRP):
                        gsl = slice(g * 128, (g + 1) * 128)
                        pt3 = ptr.tile([128, 128], F16, name="pt3", tag="tr")
                        nc.tensor.transpose(pt3[:], f0T[:, gsl], identh_t[:])
                        rr = wp.tile([128, F + 1], F16, name="rr", tag="rr")
                        nc.vector.tensor_copy(rr[:, 0:F], pt3[:])
                        nc.vector.tensor_copy(rr[:, F : F + 1],
                                              validh_t[:, g : g + 1])
                        nc.tensor.matmul(pr[:], og_t[:, g, :], rr[:],
                                         start=(g == 0), stop=(g == GRP - 1))

            # ---- readout: all-reduce partial sums, mean, classify ----
            ro_t = cp.tile([128, F + 1], FP)
            nc.vector.tensor_copy(ro_t[:], pr[:])
            nc.sync.dma_start(ar_in[:, :], ro_t[:])
            nc.gpsimd.collective_compute(
                "AllReduce", mybir.AluOpType.add, replica_groups=RG,
                ins=[ar_in[:, :].opt()], outs=[ar_out[:, :].opt()])
            nc.sync.dma_start(ro2_t[:], ar_out[:, :])
            nc.vector.tensor_scalar_max(cnt_t[:], ro2_t[:, F : F + 1], 1.0)
            nc.vector.reciprocal(rcp_t[:], cnt_t[:])
            nc.vector.tensor_tensor(hgh_t[:], ro2_t[:, 0:F],
                                    rcp_t[:].broadcast_to([128, F]), mul)
            ptf = ptr.tile([128, 128], F16, name="ptf", tag="tr")
            nc.tensor.transpose(ptf[:], hgh_t[:], identh_t[:])
            nc.vector.tensor_copy(hgT_t[:], ptf[:])
            plog = pro.tile([128, CLASSES], FP, name="plog", tag="ro")
            nc.tensor.matmul(plog[:], hgT_t[:],
                             wall_t[:, 0, (HLAYERS + 1) * F :],
                             start=True, stop=True)
            nc.vector.tensor_tensor(logit_t[:], plog[:], bcr,
                                    mybir.AluOpType.add)
            nc.sync.dma_start(out_d[:, :], logit_t[:])

    nc.finalize()
    return nc


def _make_in_maps(x, src, dst, graph_ids, Ws, bs, Wc, bc):
    deg = np.bincount(np.asarray(dst).astype(np.int64), minlength=N)
    norm = np.where(deg < 1, 1.0, deg).astype(np.float32) ** -0.5
    gs, slot_cols, nslot_cols, supers, NCH, TOT = _prep_edges(src, dst, norm)

    x = np.asarray(x, np.float32)
    graph_ids = np.asarray(graph_ids, np.int64)
    wpk = np.zeros(((HOPS + 1) * F, NW), np.float16)
    for l in range(HLAYERS + 1):
        wpk[:, l * F : (l + 1) * F] = np.asarray(Ws[l], np.float32)
    wpk[0:F, (HLAYERS + 1) * F :] = np.asarray(Wc, np.float32)
    bcol = np.stack([np.asarray(b, np.float32) for b in bs], 1)
    bcr = np.tile(np.asarray(bc, np.float32)[None, :], (128, 1))

    in_maps = []
    for c in range(NCORES):
        xl = np.zeros((NPAD, F), np.float16)
        xl[:PER] = x[c * PER : (c + 1) * PER]
        nrm = np.ones(NPAD, np.float32)
        nrm[:PER] = norm[c * PER : (c + 1) * PER]
        gsl = np.full(NPAD, -1.0, np.float32)
        gsl[:PER] = graph_ids[c * PER : (c + 1) * PER]
        colf = np.zeros((128, 2 * GRP + 3 + CLASSES), np.float32)
        colf[:, 0:GRP] = nrm.reshape(GRP, 128).T
        colf[:, GRP : 2 * GRP] = gsl.reshape(GRP, 128).T
        colf[:, 2 * GRP : 2 * GRP + 3] = bcol
        colf[:, 2 * GRP + 3 :] = bcr
        in_maps.append(dict(x_h=xl, gs_idx=gs[c], slot_h=slot_cols[c],
                            nslot_h=nslot_cols[c], colf=colf, wpk=wpk))
    return in_maps, supers, NCH, TOT


def kernel(x, src, dst, graph_ids, W0, b0, W1, b1, W2, b2, Wc, bc, **_):
    in_maps, supers, NCH, TOT = _make_in_maps(
        x, src, dst, graph_ids, [W0, W1, W2], [b0, b1, b2], Wc, bc)
    nc = _build_program(supers, NCH, TOT)
    res = bass_utils.run_bass_kernel_spmd(nc, in_maps, core_ids=list(range(NCORES)))
    return np.asarray(res.results[0]["out"], np.float32)


# revision 4
# speedup vs baseline: 2.9164x; 1.2090x over previous
"""TAGConv GNN classifier on 8 Trainium2 NeuronCores.

Sharding: nodes split into 8 contiguous slices (6250/core, padded to 6272);
edges live on the core that owns their dst. The fp16 node table (256B rows)
is allgathered into Shared DRAM each hop; each core gathers its edges' src
rows with super-batched dma_gathers (chunks grouped by dst-group, src-
sorted) and segment-sums them with per-chunk one-hot matmuls on TensorE.
Swapped matmul operands emit the aggregate feature-major directly, so hop
outputs need no PE transpose. Degrees/norms are host-precomputed. Per-graph
readout partials are all-reduced; every core computes identical logits.
"""
import os

os.environ.setdefault("JAX_COMPILATION_CACHE_DIR", "/tmp/jaxcache")
os.environ.setdefault("JAX_PERSISTENT_CACHE_MIN_COMPILE_TIME_SECS", "0")
os.environ.setdefault("JAX_PERSISTENT_CACHE_MIN_ENTRY_SIZE_BYTES", "0")

import numpy as np

import concourse.bass as bass
import concourse.bacc as bacc
import concourse.mybir as mybir
import concourse.tile as tile
from concourse import bass_utils

# The env defaults above only help if jax wasn't imported yet; config.update
# works either way, making warm invocations hit the persistent compile cache.
try:
    import jax

    jax.config.update("jax_compilation_cache_dir",
                      os.environ["JAX_COMPILATION_CACHE_DIR"])
    jax.config.update("jax_persistent_cache_min_compile_time_secs", 0.0)
    jax.config.update("jax_persistent_cache_min_entry_size_bytes", 0)
except Exception:
    pass

N, E, G = 50000, 800000, 128
F = 128
CLASSES = 10
HOPS, HLAYERS = 2, 2         # 3 TAGConv layers total
NCORES = 8


def configure(n, e, csup=96):
    """Derived sizes; module-level so debug harnesses can shrink the problem."""
    global N, E, PER, GRP, NPAD, NT, HALF, CSUP
    N, E = n, e
    PER = N // NCORES            # real nodes per core
    GRP = (PER + 127) // 128     # dst groups of 128 per core
    NPAD = GRP * 128             # padded nodes per core
    NT = NCORES * NPAD           # padded total
    HALF = NT // 2               # int16-safe split of the node table
    CSUP = csup                  # max chunks per super-gather


configure(N, E)

F16 = mybir.dt.float16
FP = mybir.dt.float32
I16 = mybir.dt.int16

NW = (HLAYERS + 1) * F + CLASSES   # packed weight columns


def _prep_edges(src, dst, norm):
    """Chunk edges by (dst-group, table-half), src-sorted, batched into
    supers of whole groups with A-chunks then B-chunks contiguous.
    nslot carries norm[dst] per edge (0 on pads) so the one-hot can be
    pre-scaled by the dst-side norm."""
    src = np.asarray(src).astype(np.int64)
    dst = np.asarray(dst).astype(np.int64)
    core = dst // PER
    local = dst - core * PER
    grp = local // 128
    slot = local % 128
    ps = (src // PER) * NPAD + (src % PER)      # padded global src id
    hi = (ps >= HALF).astype(np.int64)

    key = (core * GRP + grp) * 2 + hi
    order = np.lexsort((ps, key))
    cnt = np.bincount(key, minlength=NCORES * GRP * 2).reshape(NCORES, GRP, 2)
    CA = (-(-cnt[:, :, 0].max(axis=0) // 128)).astype(int)   # per-group chunks
    CB = (-(-cnt[:, :, 1].max(axis=0) // 128)).astype(int)

    # supers: consecutive whole groups, <= CSUP chunks each
    supers = []   # (c0, nchA, nchB, [(g, a0, na, b0, nb)]) ; a0/b0 local
    i = 0
    c0 = 0
    while i < GRP:
        j, tot = i, 0
        while j < GRP and tot + CA[j] + CB[j] <= CSUP:
            tot += CA[j] + CB[j]
            j += 1
        assert j > i, f"group {i} exceeds CSUP={CSUP}"
        gl, a, nchA = [], 0, int(sum(CA[i:j]))
        b = nchA
        for g in range(i, j):
            gl.append((g, a, int(CA[g]), b, int(CB[g])))
            a += int(CA[g])
            b += int(CB[g])
        supers.append((c0, nchA, int(sum(CB[i:j])), gl))
        c0 += tot
        i = j
    NCH = c0
    TOT = NCH * 128

    # global chunk offset per (group, half) following the super layout
    choff = np.zeros((GRP, 2), int)
    for (c0s, nchA, nchB, gl) in supers:
        for (g, a0, na, b0, nb) in gl:
            choff[g, 0] = c0s + a0
            choff[g, 1] = c0s + b0

    gidx = np.zeros((NCORES, TOT), np.int16)
    slotv = np.full((NCORES, TOT), -1.0, np.float16)
    nslotv = np.zeros((NCORES, TOT), np.float16)
    sp = ps[order]
    ss = slot[order]
    sn = norm[dst[order]]
    starts = np.concatenate([[0], np.cumsum(cnt.reshape(-1))]).astype(int)
    for c in range(NCORES):
        for g in range(GRP):
            for h in range(2):
                k = (c * GRP + g) * 2 + h
                n = int(cnt[c, g, h])
                s0 = starts[k]
                off = choff[g, h] * 128
                gidx[c, off : off + n] = (sp[s0 : s0 + n] - h * HALF).astype(np.int16)
                slotv[c, off : off + n] = ss[s0 : s0 + n]
                nslotv[c, off : off + n] = sn[s0 : s0 + n]

    gs = np.ascontiguousarray(gidx.reshape(NCORES, -1, 16).transpose(0, 2, 1))
    slot_cols = np.ascontiguousarray(
        slotv.reshape(NCORES, NCH, 128).transpose(0, 2, 1))
    nslot_cols = np.ascontiguousarray(
        nslotv.reshape(NCORES, NCH, 128).transpose(0, 2, 1))
    return gs, slot_cols, nslot_cols, supers, NCH, TOT


def _build_program(supers, NCH, TOT, shared_tables=True):
    W16 = TOT // 16
    nc = bacc.Bacc("TRN2", target_bir_lowering=False, debug=False,
                   num_devices=NCORES)
    RG = [list(range(NCORES))]
    aspace = "Shared" if shared_tables else "Local"

    # packed per-core fp16 input: x (node-major, pre-rearranged to
    # [p, g*F]) | slot | nslot
    BIGW = GRP * F + 2 * NCH
    big_d = nc.dram_tensor("big_h", [128, BIGW], F16, kind="ExternalInput")
    gs_d = nc.dram_tensor("gs_idx", [16, W16], I16, kind="ExternalInput")
    colf_d = nc.dram_tensor("colf", [128, 2 * GRP + 3 + CLASSES], FP,
                            kind="ExternalInput")
    wpk_d = nc.dram_tensor("wpk", [(HOPS + 1) * F, NW], F16, kind="ExternalInput")
    out_d = nc.dram_tensor("out", [G, CLASSES], FP, kind="ExternalOutput")

    mul = mybir.AluOpType.mult

    with tile.TileContext(nc) as tc:
        with (
            tc.tile_pool(name="const", bufs=1) as cp,
            tc.tile_pool(name="work", bufs=2) as wp,
            tc.tile_pool(name="psmm", bufs=3, space="PSUM") as pmm,
            tc.tile_pool(name="pstr", bufs=2, space="PSUM") as ptr,
            tc.tile_pool(name="psro", bufs=2, space="PSUM") as pro,
            tc.tile_pool(name="dram", bufs=1, space="DRAM") as dp,
        ):
            gidx_t = cp.tile([128, W16], I16)
            big_t = cp.tile([128, GRP * F + 2 * NCH], F16)
            colf_t = cp.tile([128, 2 * GRP + 3 + CLASSES], FP)
            wall_t = cp.tile([128, HOPS + 1, NW], F16)
            normh_t = cp.tile([128, GRP], F16)
            gsloth_t = cp.tile([128, GRP], F16)
            validh_t = cp.tile([128, GRP], F16)
            iotah_t = cp.tile([128, 128], F16)
            identh_t = cp.tile([128, 128], F16)
            og_t = cp.tile([128, GRP, 128], F16)
            tn_t = cp.tile([128, GRP, F], F16)
            f0T = cp.tile([128, NPAD], F16)
            f1T = cp.tile([128, NPAD], F16)
            f2T = cp.tile([128, NPAD], F16)
            ro2_t = cp.tile([128, F + 1], FP)
            cnt_t = cp.tile([128, 1], FP)
            rcp_t = cp.tile([128, 1], FP)
            hgh_t = cp.tile([128, F], F16)
            hgT_t = cp.tile([128, 128], F16)
            logit_t = cp.tile([128, CLASSES], FP)

            # Shared DRAM tensors may be written by only one instruction:
            # one table per AllGather (t0 + per-layer hop1 + 2 layer outs).
            Ts = [dp.tile([NT, F], F16, addr_space=aspace, name=f"T{i}")
                  for i in range(2 * HLAYERS + HOPS)]
            ag_in = dp.tile([NPAD, F], F16)
            ar_in = dp.tile([128, F + 1], FP)
            ar_out = dp.tile([128, F + 1], FP)

            rearr = lambda ap: ap.rearrange("(g p) f -> p g f", p=128)

            # ---- constants ----
            for k in range(8):
                nc.sync.dma_start(gidx_t[16 * k : 16 * (k + 1), :], gs_d[:, :])
            nc.sync.dma_start(big_t[:], big_d[:, :])
            nc.sync.dma_start(colf_t[:], colf_d[:, :])
            for k in range(HOPS + 1):
                nc.sync.dma_start(wall_t[:, k, :], wpk_d[k * 128 : (k + 1) * 128, :])

            xt = big_t[:, 0 : GRP * F].rearrange("p (g f) -> p g f", f=F)
            slot_t = big_t[:, GRP * F : GRP * F + NCH]
            nslot_t = big_t[:, GRP * F + NCH : GRP * F + 2 * NCH]
            normc = colf_t[:, 0:GRP]
            gslot = colf_t[:, GRP : 2 * GRP]
            b_t = colf_t[:, 2 * GRP : 2 * GRP + 3]
            bcr = colf_t[:, 2 * GRP + 3 : 2 * GRP + 3 + CLASSES]

            nc.gpsimd.iota(iotah_t[:], pattern=[[1, 128]], base=0,
                           channel_multiplier=0,
                           allow_small_or_imprecise_dtypes=True)
            icolh_t = cp.tile([128, 1], F16)
            nc.gpsimd.iota(icolh_t[:], pattern=[[0, 1]], base=0,
                           channel_multiplier=1,
                           allow_small_or_imprecise_dtypes=True)
            nc.vector.tensor_tensor(identh_t[:],
                                    icolh_t[:].broadcast_to([128, 128]),
                                    iotah_t[:], mybir.AluOpType.is_equal)
            nc.vector.tensor_copy(normh_t[:], normc)
            nc.vector.tensor_copy(gsloth_t[:], gslot)
            nc.vector.tensor_scalar_add(validh_t[:], gsloth_t[:], 1.0)
            nc.vector.tensor_scalar_min(validh_t[:], validh_t[:], 1.0)
            nc.vector.tensor_tensor(
                og_t[:],
                gsloth_t[:].unsqueeze(2).broadcast_to([128, GRP, 128]),
                iotah_t[:].unsqueeze(1).broadcast_to([128, GRP, 128]),
                mybir.AluOpType.is_equal)

            def hop(Tsrc, fT, make_table):
                """gather supers -> one-hot segment-sum (feature-major) -> fT;
                optionally also stage the next node table into tn_t."""
                for (c0, nchA, nchB, gl) in supers:
                    nch = nchA + nchB
                    vb = wp.tile([128, CSUP, F], F16, name="vb", tag="vb")
                    if nchA:
                        nc.gpsimd.dma_gather(
                            vb[:, 0:nchA, :], Tsrc[:, :],
                            gidx_t[:, c0 * 8 : (c0 + nchA) * 8],
                            nchA * 128, nchA * 128, F, single_packet=False)
                    if nchB:
                        nc.gpsimd.dma_gather(
                            vb[:, nchA:nch, :], Tsrc[HALF:, :],
                            gidx_t[:, (c0 + nchA) * 8 : (c0 + nch) * 8],
                            nchB * 128, nchB * 128, F, single_packet=False)
                    oh = wp.tile([128, CSUP, 128], F16, name="oh", tag="oh")
                    nc.vector.tensor_tensor(
                        oh[:, 0:nch, :],
                        slot_t[:, c0 : c0 + nch].unsqueeze(2)
                              .broadcast_to([128, nch, 128]),
                        iotah_t[:].unsqueeze(1).broadcast_to([128, nch, 128]),
                        mybir.AluOpType.is_equal)
                    nc.vector.tensor_tensor(
                        oh[:, 0:nch, :], oh[:, 0:nch, :],
                        nslot_t[:, c0 : c0 + nch].unsqueeze(2)
                               .broadcast_to([128, nch, 128]), mul)
                    for (g, a0, na, b0, nb) in gl:
                        gsl = slice(g * 128, (g + 1) * 128)
                        chunks = list(range(a0, a0 + na)) + \
                                 list(range(b0, b0 + nb))
                        if not chunks:
                            nc.vector.memset(fT[:, gsl], 0.0)
                            continue
                        psT = pmm.tile([128, 128], FP, name="psT", tag="mm")
                        for ci, c in enumerate(chunks):
                            nc.tensor.matmul(psT[:], vb[:, c, :], oh[:, c, :],
                                             start=(ci == 0),
                                             stop=(ci == len(chunks) - 1))
                        nc.vector.tensor_copy(fT[:, gsl], psT[:])
                if make_table:
                    for g in range(GRP):
                        gsl = slice(g * 128, (g + 1) * 128)
                        pt = ptr.tile([128, 128], F16, name="pt", tag="tr")
                        nc.tensor.transpose(pt[:], fT[:, gsl], identh_t[:])
                        nc.vector.tensor_tensor(
                            tn_t[:, g, :], pt[:],
                            normh_t[:, g : g + 1].broadcast_to([128, F]), mul)

            def emit_table(Tdst):
                nc.sync.dma_start(rearr(ag_in[:, :]), tn_t[:])
                nc.gpsimd.collective_compute(
                    "AllGather", mybir.AluOpType.bypass, replica_groups=RG,
                    ins=[ag_in[:, :].opt()], outs=[Tdst[:, :].opt()])

            # ---- t0: table = x*norm, f0T = x^T ----
            nc.vector.tensor_tensor(
                tn_t[:], xt[:],
                normh_t[:].unsqueeze(2).broadcast_to([128, GRP, F]), mul)
            emit_table(Ts[0])
            for g in range(GRP):
                gsl = slice(g * 128, (g + 1) * 128)
                pt = ptr.tile([128, 128], F16, name="pt0", tag="tr")
                nc.tensor.transpose(pt[:], xt[:, g, :], identh_t[:])
                nc.vector.tensor_copy(f0T[:, gsl], pt[:])

            pr = None
            for l in range(HLAYERS + 1):
                hop(Ts[2 * l], f1T, make_table=True)
                emit_table(Ts[2 * l + 1])
                hop(Ts[2 * l + 1], f2T, make_table=False)
                for g in range(GRP):
                    gsl = slice(g * 128, (g + 1) * 128)
                    ph = pmm.tile([128, 128], FP, name="ph", tag="mm")
                    for k, fk in enumerate((f0T, f1T, f2T)):
                        nc.tensor.matmul(ph[:], wall_t[:, k, l * F : (l + 1) * F],
                                         fk[:, gsl], start=(k == 0), stop=(k == 2))
                    nc.scalar.activation(f0T[:, gsl], ph[:],
                                         mybir.ActivationFunctionType.Relu,
                                         bias=b_t[:, l : l + 1])
                if l < HLAYERS:
                    for g in range(GRP):
                        gsl = slice(g * 128, (g + 1) * 128)
                        pt2 = ptr.tile([128, 128], F16, name="pt2", tag="tr")
                        nc.tensor.transpose(pt2[:], f0T[:, gsl], identh_t[:])
                        nc.vector.tensor_tensor(
                            tn_t[:, g, :], pt2[:],
                            normh_t[:, g : g + 1].broadcast_to([128, F]), mul)
                    emit_table(Ts[2 * l + 2])
                else:
                    pr = pro.tile([128, F + 1], FP, name="pr", tag="ro")
                    for g in range(GRP):
                        gsl = slice(g * 128, (g + 1) * 128)
                        pt3 = ptr.tile([128, 128], F16, name="pt3", tag="tr")
                        nc.tensor.transpose(pt3[:], f0T[:, gsl], identh_t[:])
                        rr = wp.tile([128, F + 1], F16, name="rr", tag="rr")
                        nc.vector.tensor_copy(rr[:, 0:F], pt3[:])
                        nc.vector.tensor_copy(rr[:, F : F + 1],
                                              validh_t[:, g : g + 1])
                        nc.tensor.matmul(pr[:], og_t[:, g, :], rr[:],
                                         start=(g == 0), stop=(g == GRP - 1))

            # ---- readout: all-reduce partial sums, mean, classify ----
            ro_t = cp.tile([128, F + 1], FP)
            nc.vector.tensor_copy(ro_t[:], pr[:])
            nc.sync.dma_start(ar_in[:, :], ro_t[:])
            nc.gpsimd.collective_compute(
                "AllReduce", mybir.AluOpType.add, replica_groups=RG,
                ins=[ar_in[:, :].opt()], outs=[ar_out[:, :].opt()])
            nc.sync.dma_start(ro2_t[:], ar_out[:, :])
            nc.vector.tensor_scalar_max(cnt_t[:], ro2_t[:, F : F + 1], 1.0)
            nc.vector.reciprocal(rcp_t[:], cnt_t[:])
            nc.vector.tensor_tensor(hgh_t[:], ro2_t[:, 0:F],
                                    rcp_t[:].broadcast_to([128, F]), mul)
            ptf = ptr.tile([128, 128], F16, name="ptf", tag="tr")
            nc.tensor.transpose(ptf[:], hgh_t[:], identh_t[:])
            nc.vector.tensor_copy(hgT_t[:], ptf[:])
            plog = pro.tile([128, CLASSES], FP, name="plog", tag="ro")
            nc.tensor.matmul(plog[:], hgT_t[:],
                             wall_t[:, 0, (HLAYERS + 1) * F :],
                             start=True, stop=True)
            nc.vector.tensor_tensor(logit_t[:], plog[:], bcr,
                                    mybir.AluOpType.add)
            nc.sync.dma_start(out_d[:, :], logit_t[:])

    nc.finalize()
    return nc


def _make_in_maps(x, src, dst, graph_ids, Ws, bs, Wc, bc):
    deg = np.bincount(np.asarray(dst).astype(np.int64), minlength=N)
    norm = np.where(deg < 1, 1.0, deg).astype(np.float32) ** -0.5
    gs, slot_cols, nslot_cols, supers, NCH, TOT = _prep_edges(src, dst, norm)

    x = np.asarray(x, np.float32)
    graph_ids = np.asarray(graph_ids, np.int64)
    wpk = np.zeros(((HOPS + 1) * F, NW), np.float16)
    for l in range(HLAYERS + 1):
        wpk[:, l * F : (l + 1) * F] = np.asarray(Ws[l], np.float32)
    wpk[0:F, (HLAYERS + 1) * F :] = np.asarray(Wc, np.float32)
    bcol = np.stack([np.asarray(b, np.float32) for b in bs], 1)
    bcr = np.tile(np.asarray(bc, np.float32)[None, :], (128, 1))

    in_maps = []
    for c in range(NCORES):
        xl = np.zeros((NPAD, F), np.float16)
        xl[:PER] = x[c * PER : (c + 1) * PER]
        xr = xl.reshape(GRP, 128, F).transpose(1, 0, 2).reshape(128, GRP * F)
        big = np.concatenate([xr, slot_cols[c], nslot_cols[c]], axis=1)
        nrm = np.ones(NPAD, np.float32)
        nrm[:PER] = norm[c * PER : (c + 1) * PER]
        gsl = np.full(NPAD, -1.0, np.float32)
        gsl[:PER] = graph_ids[c * PER : (c + 1) * PER]
        colf = np.zeros((128, 2 * GRP + 3 + CLASSES), np.float32)
        colf[:, 0:GRP] = nrm.reshape(GRP, 128).T
        colf[:, GRP : 2 * GRP] = gsl.reshape(GRP, 128).T
        colf[:, 2 * GRP : 2 * GRP + 3] = bcol
        colf[:, 2 * GRP + 3 :] = bcr
        in_maps.append(dict(big_h=big, gs_idx=gs[c], colf=colf, wpk=wpk))
    return in_maps, supers, NCH, TOT


def kernel(x, src, dst, graph_ids, W0, b0, W1, b1, W2, b2, Wc, bc, **_):
    in_maps, supers, NCH, TOT = _make_in_maps(
        x, src, dst, graph_ids, [W0, W1, W2], [b0, b1, b2], Wc, bc)
    nc = _build_program(supers, NCH, TOT)
    res = bass_utils.run_bass_kernel_spmd(nc, in_maps, core_ids=list(range(NCORES)))
    return np.asarray(res.results[0]["out"], np.float32)
